# revision 1
# baseline (speedup 1.0000x reference)
"""Mistral flash-attention (paged KV, GQA, sliding window) on 8 TRN2 cores.

Tensor-parallel over heads: core m owns kv-head m and q-heads 4m..4m+3,
wq/wk/wv column-sharded, wo column-sharded; the attention output (oT,
feature-major) is AllGathered in bf16 (two token-halves, pipelined with
attention and o_proj), then each core computes its 512 output columns
of o @ wo.

All on-chip layouts are feature-on-partition (transposed); the host
pre-transposes hidden_states and the windowed K-cache blocks so no
device-side transposes are needed except new-V (8 PE transposes).
Softmax skips max-subtraction (scores are bounded for this model scale);
the denominator comes from ones-vector matmuls and is applied via a
K=1 broadcast matmul + DVE reciprocal + multiply.
"""
import os
import sys
import math
import numpy as np
import ml_dtypes

import concourse.bacc as bacc
import concourse.tile as tile
from concourse import mybir
from concourse.bass_utils import run_bass_kernel_spmd

# ---- problem constants (hardcoded per contest rules) ----
HID = 4096; H = 32; KVH = 8; D = 128
B = 4; Q = 256; KV = 2048; HIST = KV - Q
BS = 64; NB = KV // BS; NBLOCKS = 160
WINDOW = 1024; THETA = 10000.0
T = B * Q                      # 1024 tokens
M = 8                          # cores
HPC = H // M                   # 4 q-heads per core
SCALE = 1.0 / math.sqrt(D)

# windowed cache key range: slots (HIST-WINDOW, HIST) come from the cache,
# slots [HIST, KV) are the new tokens computed on-chip.
K0 = HIST - WINDOW             # 768, first (masked-boundary) cache slot
NCBLK = (HIST - K0) // BS      # 16 cache blocks per seq
CKEYS = NCBLK * BS             # 1024 cache keys per seq
NKT = (CKEYS + Q) // 128       # 10 key tiles of 128 per seq
BOUND_KT = (0, 1, NKT - 2, NKT - 1)   # diagonal-masked key tiles

F32 = mybir.dt.float32
F32R = mybir.dt.float32r
BF16 = mybir.dt.bfloat16

# projection passes: two feature blocks each (q0q1, q2q3, kv)
PASSES = [("q", 0, "q", 1), ("q", 2, "q", 3), ("k", 0, "v", 0)]

_CACHE = {}


def _build():
    from contextlib import ExitStack
    nc = bacc.Bacc("TRN2", target_bir_lowering=False, debug=False,
                   enable_asserts=False, num_devices=M)

    dt_in = nc.dram_tensor
    hidT = dt_in("hidT", [HID, T], BF16, kind="ExternalInput").ap()
    # wcat[p, c] = [128, 256]: chunk c of the two feature blocks of pass p
    wcat = dt_in("wcat", [3, 32, 128, 256], BF16, kind="ExternalInput").ap()
    wo = dt_in("wo", [HID, 512], BF16, kind="ExternalInput").ap()
    kcT = dt_in("kcT", [B, 128, CKEYS], F32R, kind="ExternalInput").ap()
    vc = dt_in("vc", [B, 128, CKEYS], F32R, kind="ExternalInput").ap()
    cosT = dt_in("cosT", [128, T], F32, kind="ExternalInput").ap()
    sinTs = dt_in("sinTs", [128, T], F32, kind="ExternalInput").ap()
    masks = dt_in("masks", [len(BOUND_KT), 128, 512], F32, kind="ExternalInput").ap()
    ident = dt_in("ident", [128, 128], F32, kind="ExternalInput").ap()
    onesk = dt_in("onesk", [128, 1], F32R, kind="ExternalInput").ap()
    onesr = dt_in("onesr", [1, 128], F32, kind="ExternalInput").ap()
    outp = dt_in("out", [T, 512], F32, kind="ExternalOutput").ap()

    ag_in = [nc.dram_tensor(f"ag_in{s}", [512, 256], BF16).ap() for s in range(4)]
    ag_out = [nc.dram_tensor(f"ag_out{s}", [H * D, 256], BF16,
                             addr_space="Shared").ap() for s in range(4)]

    with tile.TileContext(nc) as tc, ExitStack() as top:
        persist = top.enter_context(tc.tile_pool(name="persist", bufs=1))

        qT = persist.tile([128, HPC * T], F32R, tag="qT")     # (head, token)
        kT = persist.tile([128, T], F32R, tag="kT")
        vnat = persist.tile([128, 8 * 128], F32R, tag="vnat")  # 8 token-tiles
        oT = persist.tile([128, HPC * T], BF16, tag="oT")
        onesk_sb = persist.tile([128, 1], F32R, tag="onesk")
        onesr_sb = persist.tile([1, 128], F32, tag="onesr")
        id_sb = persist.tile([128, 128], F32, tag="ident")
        nc.sync.dma_start(onesk_sb[:], onesk[:, :])
        nc.sync.dma_start(onesr_sb[:], onesr[:, :])
        nc.sync.dma_start(id_sb[:], ident[:, :])

        # tiny warm-up AllGather so the first real AG doesn't pay
        # first-collective overhead
        warm_in = nc.dram_tensor("warm_in", [1, 128], BF16).ap()
        warm_out = nc.dram_tensor("warm_out", [M, 128], BF16,
                                  addr_space="Shared").ap()
        nc.gpsimd.dma_start(warm_in[:, :], ident[0:1, 0:128])
        nc.gpsimd.collective_compute(
            "AllGather", mybir.AluOpType.bypass,
            replica_groups=[list(range(M))],
            ins=[warm_in.opt()], outs=[warm_out.opt()])

        # ---------------- stage 1: QKV projections + RoPE ----------------
        with tc.tile_pool(name="s1", bufs=1) as s1, \
             tc.tile_pool(name="s1psum", bufs=1, space="PSUM") as s1p, \
             tc.tile_pool(name="wstream", bufs=24) as ws, \
             tc.tile_pool(name="ropetmp", bufs=2) as rt:
            cos_sb = s1.tile([128, T], F32, tag="cos")
            sin_sb = s1.tile([128, T], F32, tag="sin")
            nc.sync.dma_start(cos_sb[:], cosT[:, :])
            nc.sync.dma_start(sin_sb[:], sinTs[:, :])
            vT = s1.tile([128, T], F32, tag="vT")

            hid = [None] * 32
            wtiles = {}
            for p in range(3):
                for c in range(32):
                    wt = ws.tile([128, 256], BF16, tag="w", name=f"w{p}_{c}")
                    nc.sync.dma_start(wt[:], wcat[p, c])
                    wtiles[(p, c)] = wt
                    if p == 0:
                        htile = s1.tile([128, T], BF16, tag=f"hid{c}",
                                        name=f"hid{c}")
                        nc.sync.dma_start(htile[:], hidT[128 * c:128 * (c + 1), :])
                        hid[c] = htile

            for p, (ka, fa, kb, fb) in enumerate(PASSES):
                accs = [s1p.tile([128, 512], F32, tag=f"acc{i}", bufs=1,
                                 name=f"acc{p}_{i}") for i in range(4)]
                for c in range(32):
                    wt = wtiles[(p, c)]
                    for i in range(4):
                        th = i % 2
                        wsl = slice(0, 128) if i < 2 else slice(128, 256)
                        nc.tensor.matmul(accs[i][:], wt[:, wsl],
                                         hid[c][:, 512 * th:512 * (th + 1)],
                                         start=(c == 0), stop=(c == 31))
                for i, (kind, f) in enumerate([(ka, fa), (ka, fa), (kb, fb), (kb, fb)]):
                    th = i % 2
                    ps = accs[i]
                    sl = slice(512 * th, 512 * (th + 1))
                    if kind == "v":
                        nc.scalar.copy(vT[:, sl], ps[:])
                        continue
                    dest = qT[:, 1024 * f + 512 * th: 1024 * f + 512 * (th + 1)] \
                        if kind == "q" else kT[:, sl]
                    t1 = rt.tile([128, 512], F32, tag="t1", name=f"t1_{p}_{i}")
                    t2 = rt.tile([128, 512], F32, tag="t2", name=f"t2_{p}_{i}")
                    nc.vector.tensor_mul(t1[:], ps[:], cos_sb[:, sl])
                    nc.vector.tensor_mul(t2[0:64, :], ps[64:128, :], sin_sb[0:64, sl])
                    nc.vector.tensor_mul(t2[64:128, :], ps[0:64, :], sin_sb[64:128, sl])
                    nc.vector.tensor_add(dest, t1[:], t2[:])

            # transpose vT -> vnat (token-major) via PE
            for tt in range(8):
                tp = s1p.tile([128, 128], F32, tag="tr", bufs=2, name=f"tp{tt}")
                nc.tensor.transpose(tp[:], vT[:, 128 * tt:128 * (tt + 1)], id_sb[:])
                nc.vector.tensor_copy(vnat[:, 128 * tt:128 * (tt + 1)], tp[:])

        # ---------------- stage 2: attention + pipelined AG + o_proj -------
        with tc.tile_pool(name="psum23", bufs=2, space="PSUM") as psum23, \
             tc.tile_pool(name="s2", bufs=1) as s2, \
             tc.tile_pool(name="es", bufs=2) as es, \
             tc.tile_pool(name="s2tmp", bufs=2) as s2t, \
             tc.tile_pool(name="s3", bufs=6) as s3, \
             tc.tile_pool(name="s3o", bufs=2) as s3o:
            wo_sb = s2.tile([128, 32 * 512], BF16, tag="wo_sb")
            kc_sb = s2.tile([128, B * CKEYS], F32R, tag="kc")
            vc_sb = s2.tile([128, B * CKEYS], F32R, tag="vc")
            mask_sb = s2.tile([128, len(BOUND_KT) * 512], F32, tag="mask")
            for b in range(B):
                nc.sync.dma_start(kc_sb[:, CKEYS * b:CKEYS * (b + 1)], kcT[b])
                nc.sync.dma_start(vc_sb[:, CKEYS * b:CKEYS * (b + 1)], vc[b])
            for i in range(len(BOUND_KT)):
                nc.sync.dma_start(mask_sb[:, 512 * i:512 * (i + 1)], masks[i])
            for c in range(32):  # prefetch wo for stage 3
                nc.sync.dma_start(wo_sb[:, 512 * c:512 * (c + 1)],
                                  wo[128 * c:128 * (c + 1), :])

            qT4 = qT[:].rearrange("p (h t) -> p h t", h=HPC)

            def attn_tail(b, hp, oTp, lp):
                l_sb = s2t.tile([1, 512], F32, tag="l", name=f"l{b}_{hp}")
                nc.vector.tensor_copy(l_sb[:], lp[:])
                rbp = psum23.tile([128, 512], F32, tag="B", bufs=3,
                                  name=f"rbp{b}_{hp}")
                nc.tensor.matmul(rbp[:], onesr_sb[:], l_sb[:])
                rb_sb = s2t.tile([128, 512], F32, tag="rb", name=f"rb{b}_{hp}")
                nc.vector.reciprocal(rb_sb[:], rbp[:])
                for i in range(2):
                    h = 2 * hp + i
                    nc.vector.tensor_mul(
                        oT[:, T * h + Q * b:T * h + Q * (b + 1)],
                        oTp[:, 256 * i:256 * (i + 1)],
                        rb_sb[:, 256 * i:256 * (i + 1)])
                    nc.sync.dma_start(
                        ag_in[b][128 * h:128 * (h + 1), :],
                        oT[:, T * h + Q * b:T * h + Q * (b + 1)])
                if hp == HPC // 2 - 1:
                    nc.gpsimd.collective_compute(
                        "AllGather", mybir.AluOpType.bypass,
                        replica_groups=[list(range(M))],
                        ins=[ag_in[b].opt()], outs=[ag_out[b].opt()])

            pending = None
            for b in range(B):
                for hp in range(HPC // 2):
                    rhs_q = qT4[:, 2 * hp:2 * hp + 2, Q * b:Q * (b + 1)]
                    expS = es.tile([128, NKT * 512], F32R, tag="expS", bufs=2,
                                   name=f"expS{b}_{hp}")
                    for kt in range(NKT):
                        if kt < NKT - 2:
                            lhs_k = kc_sb[:, CKEYS * b + 128 * kt:
                                          CKEYS * b + 128 * (kt + 1)]
                        else:
                            j = kt - (NKT - 2)
                            lhs_k = kT[:, Q * b + 128 * j:Q * b + 128 * (j + 1)]
                        sps = psum23.tile([128, 512], F32, tag="A", bufs=3,
                                          name=f"sps{b}_{hp}_{kt}")
                        nc.tensor.matmul(sps[:], lhs_k, rhs_q)
                        esl = expS[:, 512 * kt:512 * (kt + 1)]
                        nc.scalar.activation(esl, sps[:],
                                             mybir.ActivationFunctionType.Exp,
                                             scale=SCALE)
                        if kt in BOUND_KT:
                            mi = BOUND_KT.index(kt)
                            nc.vector.tensor_mul(
                                esl, esl, mask_sb[:, 512 * mi:512 * (mi + 1)])
                    if pending is not None:
                        attn_tail(*pending)
                    oTp = psum23.tile([128, 512], F32, tag="B", bufs=3,
                                      name=f"oTp{b}_{hp}")
                    lp = psum23.tile([1, 512], F32, tag="B", bufs=3,
                                     name=f"lp{b}_{hp}")
                    for kt in range(NKT):
                        if kt < NKT - 2:
                            lhs_v = vc_sb[:, CKEYS * b + 128 * kt:
                                          CKEYS * b + 128 * (kt + 1)]
                        else:
                            j = kt - (NKT - 2)
                            lhs_v = vnat[:, 128 * (2 * b + j):128 * (2 * b + j + 1)]
                        nc.tensor.matmul(oTp[:], lhs_v,
                                         expS[:, 512 * kt:512 * (kt + 1)],
                                         start=(kt == 0), stop=(kt == NKT - 1))
                    for kt in range(NKT):
                        nc.tensor.matmul(lp[:], onesk_sb[:],
                                         expS[:, 512 * kt:512 * (kt + 1)],
                                         start=(kt == 0), stop=(kt == NKT - 1))
                    pending = (b, hp, oTp, lp)
            attn_tail(*pending)

            # o_proj: one pass per 256-token quarter (gated by that AG)
            for qtr in range(4):
                out_ps = [psum23.tile([128, 512], F32, tag=t, bufs=1,
                                      name=f"outps{qtr}_{t}")
                          for t in ("C", "D")]
                for c in range(32):
                    oc = s3.tile([128, 256], BF16, tag="oc", bufs=16,
                                 name=f"oc{qtr}_{c}")
                    nc.sync.dma_start(oc[:], ag_out[qtr][128 * c:128 * (c + 1), :])
                    for t in range(2):
                        nc.tensor.matmul(out_ps[t][:],
                                         oc[:, 128 * t:128 * (t + 1)],
                                         wo_sb[:, 512 * c:512 * (c + 1)],
                                         start=(c == 0), stop=(c == 31))
                for t in range(2):
                    osb = s3o.tile([128, 512], F32, tag="os",
                                   name=f"osb{qtr}_{t}")
                    nc.vector.tensor_copy(osb[:], out_ps[t][:])
                    row = 256 * qtr + 128 * t
                    nc.sync.dma_start(outp[row:row + 128, :], osb[:])

    nc.compile()
    return nc


def _prep_inputs(hidden_states, wq, wk, wv, wo, k_cache, v_cache,
                 position_ids, q_start_loc, q_seq_length, kv_seq_length,
                 block_offsets):
    f32 = np.float32
    hidden_states = np.asarray(hidden_states, f32)
    position_ids = np.asarray(position_ids, np.int32)
    block_offsets = np.asarray(block_offsets, np.int32)

    hidT = np.ascontiguousarray(hidden_states.T).astype(ml_dtypes.bfloat16)  # [HID, T]

    # rope factors per (d, token)
    half = D // 2
    inv = 1.0 / (THETA ** (np.arange(half, dtype=f32) / half))
    f = position_ids.astype(f32)[:, None] * inv[None, :]            # [T, 64]
    cos = np.cos(f); sin = np.sin(f)
    cosT = np.ascontiguousarray(np.concatenate([cos, cos], 1).T)    # [128, T]
    sinTs = np.ascontiguousarray(np.concatenate([-sin, sin], 1).T)  # [128, T]

    # boundary masks [4, 128, 512] (two identical 256-col halves per head pair)
    qpos = HIST + np.arange(Q)
    m4 = np.empty((len(BOUND_KT), 128, 512), f32)
    for i, kt in enumerate(BOUND_KT):
        kpos = K0 + 128 * kt + np.arange(128)
        valid = ((kpos[:, None] <= qpos[None, :]) &
                 (kpos[:, None] > qpos[None, :] - WINDOW)).astype(f32)
        m4[i] = np.concatenate([valid, valid], 1)
    for kt in range(NKT):          # non-boundary tiles must be fully valid
        if kt in BOUND_KT:
            continue
        kpos = K0 + 128 * kt + np.arange(128)
        assert ((kpos[:, None] <= qpos[None, :]) &
                (kpos[:, None] > qpos[None, :] - WINDOW)).all()

    ident = np.eye(128, dtype=f32)
    onesk = np.ones((128, 1), f32)
    onesr = np.ones((1, 128), f32)

    blk0 = K0 // BS
    in_maps = []
    for m in range(M):
        wq_m = np.asarray(wq[:, 512 * m:512 * (m + 1)], f32)
        wk_m = np.asarray(wk[:, 128 * m:128 * (m + 1)], f32)
        wv_m = np.asarray(wv[:, 128 * m:128 * (m + 1)], f32)
        # feature blocks in pass order: (q0,q1), (q2,q3), (k,v)
        fblocks = [wq_m[:, 0:128], wq_m[:, 128:256],
                   wq_m[:, 256:384], wq_m[:, 384:512], wk_m, wv_m]
        wcat = np.empty((3, 32, 128, 256), ml_dtypes.bfloat16)
        for p in range(3):
            a = fblocks[2 * p].reshape(32, 128, 128)
            bb = fblocks[2 * p + 1].reshape(32, 128, 128)
            wcat[p] = np.concatenate([a, bb], axis=2)
        wo_m = np.asarray(wo[:, 512 * m:512 * (m + 1)], f32).astype(ml_dtypes.bfloat16)

        kcT_m = np.empty((B, 128, CKEYS), f32)
        vc_m = np.empty((B, 128, CKEYS), f32)
        for b in range(B):
            blks = block_offsets[b, blk0:blk0 + NCBLK]
            kc = np.asarray(k_cache[blks, :, m, :], f32)     # [16, 64, 128]
            vcb = np.asarray(v_cache[blks, :, m, :], f32)
            kcT_m[b] = kc.reshape(CKEYS, 128).T              # [128 d, keys]
            vc_m[b] = vcb.reshape(8, 128, 128).transpose(1, 0, 2).reshape(128, CKEYS)
        in_maps.append(dict(
            hidT=hidT, wcat=wcat, wo=wo_m,
            kcT=np.ascontiguousarray(kcT_m), vc=np.ascontiguousarray(vc_m),
            cosT=cosT, sinTs=sinTs, masks=m4, ident=ident,
            onesk=onesk, onesr=onesr))
    return in_maps


def kernel(**inputs):
    in_maps = _prep_inputs(**inputs)
    if "nc" not in _CACHE:
        _CACHE["nc"] = _build()
    nc = _CACHE["nc"]

    kwargs = {}
    if os.environ.get("KERNEL_TRACE"):
        import types as _types
        from trn_agent_boot.trn_boot import _ntff_profile_via_ctypes
        hook = _ntff_profile_via_ctypes('/opt/axon/libaxon_pjrt.so')
        mod = _types.ModuleType("antenv.axon_hooks")
        mod.get_axon_ntff_profile_hook = lambda: hook
        sys.modules["antenv.axon_hooks"] = mod
        tdir = os.environ.get("KERNEL_TRACE_DIR", "/tmp/kernel_trace")
        os.makedirs(tdir, exist_ok=True)
        kwargs = dict(trace=True, tmpdir=tdir)

    res = run_bass_kernel_spmd(nc, in_maps, core_ids=list(range(M)), **kwargs)
    if res.exec_time_ns is not None:
        print(f"HW exec time: {res.exec_time_ns} ns")
    out = np.concatenate([res.results[m]["out"] for m in range(M)], axis=1)
    return np.ascontiguousarray(out, np.float32)

